# revision 14
# baseline (speedup 1.0000x reference)
"""CASVDDenseMul fused kernel for 8 Trainium2 NeuronCores.

Reference computation (fp32):
    chi = sigmoid(context @ W + B)          # [B, R]
    t   = (inputs @ U) * (S * chi)          # [B, R]
    out = relu(t @ V.T + 2*bias)            # [B, UNITS]

Sharding: data-parallel over batch; each of the 8 cores handles 512 rows.
All factor weights (U, S, V, W, B, bias) are replicated.

Design notes (v3 -- PE-dense):
  - Everything travels as bf16 (the PE runs one element/cell/cycle for
    any dtype, so bf16 matmuls at fp32r speed while halving DMA bytes;
    rel-err ~3e-3 vs the 2e-2 gate). S is folded into U's columns on the
    host.
  - The per-core PE work (~29us of matmul streaming) exceeds the input
    stream time (~22us at ~410GB/s), so the kernel is shaped to keep the
    PE gapless: U/x stream in 0.26MB pieces so mm1 starts ~1.3us after
    the first descriptor lands; chi fills mm1's DMA-wait gaps; the
    sub-block-B mm1 is interleaved with sub-block-A's V-matmul waves; VT
    streams mid-stream and is consumed piece-by-piece as the moving
    operand of mm2 (t' is stationary), which also lands the output in
    natural [batch, units] orientation.
  - mm2 uses 1024-wide bf16 moving operands (one accumulation group per
    2-bank PSUM tile), relu-evacuated alternately by ACT and DVE, with
    per-wave 0.26MB output writes so the write tail after the last
    matmul is short.
  - PSUM note: start=True clears has_written BANK-wide, so only the
    first matmul into a shared bank carries start=True (mm1's second
    rank-half group relies on the cleared bits to overwrite on its first
    accumulation step).
"""

import numpy as np
import ml_dtypes

from concourse import bacc, mybir
from concourse import tile
from concourse.bass_utils import run_bass_kernel_spmd

N_CORES = 8
B_SZ, N_IN, N_CTX, UNITS, RANK = 4096, 4096, 512, 4096, 256
BS = B_SZ // N_CORES   # 512 batch rows per core

P = 128
KC_IN = N_IN // P      # 32 contraction chunks for x @ U
KC_CTX = N_CTX // P    # 4  contraction chunks for ctx @ W
RT = RANK // P         # 2  rank tiles
NQ = 8                 # U/x stream pieces
KPQ = KC_IN // NQ      # 4 chunks per piece
NSB = 2                # batch sub-blocks
BSB = BS // NSB        # 256 batch cols per sub-block
NBT = BSB // P         # 2 batch tiles (128) per sub-block
NW = 4                 # VT pieces / unit waves (1024 units each)
WU = UNITS // NW       # 1024 units per wave

BF16 = mybir.dt.bfloat16
FP32 = mybir.dt.float32
FP32R = mybir.dt.float32r

bf16 = ml_dtypes.bfloat16


def _build_nc(use_b, use_bias):
    nc = bacc.Bacc("TRN2", target_bir_lowering=False, debug=False, enable_asserts=False)

    wctx = nc.declare_dram_parameter("wctx", [P, KC_CTX * (RANK + BS)], BF16, isOutput=False)
    u8 = nc.declare_dram_parameter("u8", [NQ, P, KPQ * RANK], BF16, isOutput=False)
    xg = nc.declare_dram_parameter("xg", [NSB, NQ, P, KPQ * BSB], BF16, isOutput=False)
    vt4 = nc.declare_dram_parameter("vt4", [NW, P, RT * WU], BF16, isOutput=False)
    if use_b:
        bvec = nc.declare_dram_parameter("bvec", [P, RT], FP32, isOutput=False)
    if use_bias:
        brow = nc.declare_dram_parameter("brow", [1, P + UNITS], FP32R, isOutput=False)
    out_d = nc.declare_dram_parameter("out_d", [BS, UNITS], BF16, isOutput=True)

    with tile.TileContext(nc) as tc:
        with (
            tc.tile_pool(name="small", bufs=1) as small,
            tc.tile_pool(name="stream", bufs=1) as stream,
            tc.tile_pool(name="acts", bufs=1) as acts,
            tc.tile_pool(name="ostage", bufs=6) as ostage,
            tc.tile_pool(name="pchi", bufs=1, space="PSUM") as pchi,
            tc.tile_pool(name="pt", bufs=1, space="PSUM") as pt,
            tc.tile_pool(name="pout", bufs=2, space="PSUM") as pout,
        ):
            # ---- SBUF tiles ----
            wctx_sb = small.tile([P, KC_CTX * (RANK + BS)], BF16, tag="wctx")
            u_sb = small.tile([P, NQ, KPQ * RANK], BF16, tag="u")
            x_sb = [[stream.tile([P, KPQ * BSB], BF16, tag=f"x{s}{g}", name=f"x{s}{g}")
                     for g in range(NQ)] for s in range(NSB)]
            vt_sb = small.tile([P, NW, RT * WU], BF16, tag="vt")
            if use_b:
                bvec_sb = small.tile([P, RT], FP32, tag="bvec")
            if use_bias:
                brow_sb = small.tile([1, P + UNITS], FP32R, tag="brow")
            s_chi = acts.tile([P, RT, BS], FP32, tag="schi")
            t_sb = [acts.tile([P, RT, BSB], BF16, tag=f"tsb{s}", name=f"tsb{s}")
                    for s in range(NSB)]
            junk = acts.tile([P, P], BF16, tag="junk")

            # ---- DMA issue queues (per-ring order == consumption order).
            # u/xA piece-pairs alternate rings so mm1-A piece k needs only
            # the k-th completion on each ring; wctx mid-stream (chi is gap
            # filler); xB next so mm1-B continues the PE's stream-paced run;
            # VT LAST -- by then the PE has ~17us of mm2 work left, which
            # runs dense against VT's 5.5us arrival.
            nc.sync.dma_start(u_sb[:, 0, :], u8[0])
            nc.scalar.dma_start(x_sb[0][0][:], xg[0, 0])
            nc.sync.dma_start(x_sb[0][1][:], xg[0, 1])
            nc.scalar.dma_start(u_sb[:, 1, :], u8[1])
            nc.sync.dma_start(u_sb[:, 2, :], u8[2])
            nc.scalar.dma_start(x_sb[0][2][:], xg[0, 2])
            nc.sync.dma_start(x_sb[0][3][:], xg[0, 3])
            nc.scalar.dma_start(u_sb[:, 3, :], u8[3])
            if use_b:
                nc.sync.dma_start(bvec_sb[:], bvec[:])
            if use_bias:
                nc.sync.dma_start(brow_sb[:], brow[:])
            nc.scalar.dma_start(wctx_sb[:], wctx[:])
            nc.sync.dma_start(u_sb[:, 4, :], u8[4])
            nc.scalar.dma_start(x_sb[0][4][:], xg[0, 4])
            nc.sync.dma_start(x_sb[0][5][:], xg[0, 5])
            nc.scalar.dma_start(u_sb[:, 5, :], u8[5])
            nc.sync.dma_start(u_sb[:, 6, :], u8[6])
            nc.scalar.dma_start(x_sb[0][6][:], xg[0, 6])
            nc.sync.dma_start(x_sb[0][7][:], xg[0, 7])
            nc.scalar.dma_start(u_sb[:, 7, :], u8[7])
            for g in range(NQ):
                eng = nc.scalar if g % 2 == 0 else nc.sync
                eng.dma_start(x_sb[1][g][:], xg[1, g])
            nc.scalar.dma_start(vt_sb[:, 0, :], vt4[0])
            nc.sync.dma_start(vt_sb[:, 1, :], vt4[1])
            nc.scalar.dma_start(vt_sb[:, 2, :], vt4[2])
            nc.sync.dma_start(vt_sb[:, 3, :], vt4[3])

            # ---- PE warm-up: keep the HAM activity window busy from t=0
            # so the clock gate lifts to 2.4 GHz before the real stream.
            nc.gpsimd.memset(junk[:], 0.0)
            warm_ps = pchi.tile([P, BS], FP32, tag="chi", name="warm_ps")
            for _ in range(16):
                nc.tensor.matmul(
                    warm_ps[:, :P], junk[:], junk[:],
                    start=True, stop=True, skip_group_check=True,
                )

            # ---- chi' = sigmoid(W.T @ ctxT + B)  (S folded into U) ----
            for rt in range(RT):
                psum_chi = pchi.tile([P, BS], FP32, tag="chi", name=f"pchi{rt}")
                for k in range(KC_CTX):
                    base = k * (RANK + BS)
                    nc.tensor.matmul(
                        psum_chi[:],
                        wctx_sb[:, base + rt * P: base + (rt + 1) * P],
                        wctx_sb[:, base + RANK: base + RANK + BS],
                        start=(k == 0), stop=(k == KC_CTX - 1),
                        skip_group_check=True,
                    )
                nc.scalar.activation(
                    s_chi[:, rt, :], psum_chi[:],
                    mybir.ActivationFunctionType.Sigmoid,
                    bias=(bvec_sb[:, rt:rt + 1] if use_b else 0.0), scale=1.0,
                )

            psum_t = [pt.tile([P, RT * BSB], FP32, tag=f"pt{s}", name=f"pt{s}")
                      for s in range(NSB)]

            def emit_mm1_piece(s, q):
                # psum_t[s][:, rt*BSB:(rt+1)*BSB] += U'_k.T @ x_k, k in piece q.
                # Both rank-half groups share ONE psum bank; start=True clears
                # has_written BANK-wide, so only the very first matmul carries
                # it (the rt1 group's k=0 lands on cleared bits and start=False
                # already overwrites).
                for j in range(KPQ):
                    k = q * KPQ + j
                    for rt in range(RT):
                        nc.tensor.matmul(
                            psum_t[s][:, rt * BSB:(rt + 1) * BSB],
                            u_sb[:, q, j * RANK + rt * P: j * RANK + (rt + 1) * P],
                            x_sb[s][q][:, j * BSB:(j + 1) * BSB],
                            start=(k == 0 and rt == 0),
                            stop=(k == KC_IN - 1),
                            skip_group_check=True,
                        )

            def emit_tprime(s):
                for rt in range(RT):
                    nc.vector.tensor_mul(
                        t_sb[s][:, rt, :],
                        psum_t[s][:, rt * BSB:(rt + 1) * BSB],
                        s_chi[:, rt, s * BSB:(s + 1) * BSB],
                    )

            def emit_mm2_wave(s, w, bt, widx):
                # out[bt-rows, wave-units] = t'.T @ VT (+ 2*bias), relu, DMA.
                # One accumulation group per 2-bank tile (N=1024 moving).
                pw = pout.tile([P, WU], FP32, tag="po")
                for rt in range(RT):
                    for h in range(2):
                        nc.tensor.matmul(
                            pw[:, h * 512:(h + 1) * 512],
                            t_sb[s][:, rt, bt * P:(bt + 1) * P],
                            vt_sb[:, w, rt * WU + h * 512: rt * WU + (h + 1) * 512],
                            start=(rt == 0),
                            stop=(rt == RT - 1 and not use_bias),
                            skip_group_check=True,
                        )
                if use_bias:
                    for h in range(2):
                        nc.tensor.matmul(
                            pw[:, h * 512:(h + 1) * 512],
                            brow_sb[:, 0:P],
                            brow_sb[:, P + w * WU + h * 512: P + w * WU + (h + 1) * 512],
                            start=False, stop=True,
                            skip_group_check=True,
                        )
                o_sb = ostage.tile([P, WU], BF16, tag="osb")
                # split the evacuation across ACT and DVE so the PSUM banks
                # free ~2x sooner (the next-next wave's matmuls wait on this)
                nc.scalar.activation(
                    o_sb[:, 0:512], pw[:, 0:512],
                    mybir.ActivationFunctionType.Relu,
                )
                nc.vector.tensor_scalar(
                    o_sb[:, 512:WU], pw[:, 512:WU], 0.0, None,
                    op0=mybir.AluOpType.max,
                )
                rows = slice(s * BSB + bt * P, s * BSB + (bt + 1) * P)
                cols = slice(w * WU, (w + 1) * WU)
                eng = nc.scalar if widx % 2 == 0 else nc.sync
                eng.dma_start(out_d[rows, cols], o_sb[:])

            # mm1-A then mm1-B ride the DMA-paced stream (chi fills gaps);
            # all of mm2 then runs dense, paced only by VT's arrival.
            for q in range(NQ):
                emit_mm1_piece(0, q)
            emit_tprime(0)
            for q in range(NQ):
                emit_mm1_piece(1, q)
            emit_tprime(1)
            widx = 0
            for w in range(NW):
                for s in range(NSB):
                    for bt in range(NBT):
                        emit_mm2_wave(s, w, bt, widx)
                        widx += 1

    nc.finalize()
    return nc


_NC_CACHE = {}


def _get_nc(use_b=False, use_bias=False):
    key = (use_b, use_bias)
    if key not in _NC_CACHE:
        _NC_CACHE[key] = _build_nc(use_b, use_bias)
    return _NC_CACHE[key]


def _round_fp32r(a):
    """Round fp32 to the fp32r grid (11-bit mantissa; low 12 bits zero)."""
    u = np.ascontiguousarray(a, dtype=np.float32).view(np.uint32)
    r = (u + np.uint32(0x7FF) + ((u >> np.uint32(12)) & np.uint32(1))) & np.uint32(0xFFFFF000)
    return r.view(np.float32)


def build(inputs, context, U, S, V, W, B, bias):
    """Host-side packing: returns (nc, in_maps)."""
    use_b = bool(np.any(np.asarray(B)))
    use_bias = bool(np.any(np.asarray(bias)))

    # U with S folded into its columns, chunked for the stream:
    # u8[q, p, j*RANK + r] = (U*S)[(q*KPQ+j)*128 + p, r]
    US = (np.asarray(U, np.float32) * np.asarray(S, np.float32)[None, :]).astype(bf16)
    u8 = np.ascontiguousarray(
        US.reshape(NQ, KPQ, P, RANK).transpose(0, 2, 1, 3).reshape(NQ, P, KPQ * RANK)
    )

    # VT pieces: vt4[c, p, rt*WU + m'] = V.T[rt*128 + p, c*WU + m']
    VTb = np.asarray(V, np.float32).T.astype(bf16)          # [RANK, UNITS]
    vt4 = np.ascontiguousarray(
        VTb.reshape(RT, P, NW, WU).transpose(2, 1, 0, 3).reshape(NW, P, RT * WU)
    )

    Wk = np.asarray(W, np.float32).astype(bf16).reshape(KC_CTX, P, RANK)
    ctxT = np.asarray(context, np.float32).astype(bf16).T   # [N_CTX, B_SZ]
    xT = np.asarray(inputs, np.float32).astype(bf16).T      # [N_IN, B_SZ]

    bvec = np.ascontiguousarray(np.asarray(B, np.float32).reshape(RT, P).T)
    brow = np.empty((1, P + UNITS), np.float32)
    brow[0, :P] = 1.0
    brow[0, P:] = 2.0 * np.asarray(bias, np.float32)
    brow = _round_fp32r(brow)

    in_maps = []
    for c in range(N_CORES):
        sl = slice(c * BS, (c + 1) * BS)
        # wctx[p, k*(RANK+BS) + ...] = [W_k | ctx_k] per contraction chunk
        wc = np.empty((KC_CTX, P, RANK + BS), bf16)
        wc[:, :, :RANK] = Wk
        wc[:, :, RANK:] = ctxT[:, sl].reshape(KC_CTX, P, BS)
        wctx = np.ascontiguousarray(
            wc.transpose(1, 0, 2).reshape(P, KC_CTX * (RANK + BS))
        )
        # xg[s, g, p, j*BSB + b'] = xT[(g*KPQ+j)*128 + p, c*BS + s*BSB + b']
        xc = xT[:, sl]                                       # [N_IN, BS]
        xgc = np.ascontiguousarray(
            xc.reshape(NQ, KPQ, P, NSB, BSB)
              .transpose(3, 0, 2, 1, 4)
              .reshape(NSB, NQ, P, KPQ * BSB)
        )
        m = {"wctx": wctx, "u8": u8, "xg": xgc, "vt4": vt4}
        if use_b:
            m["bvec"] = bvec
        if use_bias:
            m["brow"] = brow
        in_maps.append(m)
    return _get_nc(use_b, use_bias), in_maps


def gather_out(results):
    out = np.empty((B_SZ, UNITS), dtype=np.float32)
    for c in range(N_CORES):
        out[c * BS:(c + 1) * BS, :] = results[c]["out_d"].astype(np.float32)
    return out


def kernel(inputs, context, U, S, V, W, B, bias):
    nc, in_maps = build(inputs, context, U, S, V, W, B, bias)
    res = run_bass_kernel_spmd(nc, in_maps, list(range(N_CORES)))
    return gather_out(res.results)
